# revision 8
# baseline (speedup 1.0000x reference)
"""Masked dot-product attention on 8 Trainium2 NeuronCores.

Full inputs: queries/keys/values [8, 2048, 128] f32, valid_lens [8] i32.
Output: softmax(Q K^T / sqrt(128), masked to valid_lens) @ V, [8, 2048, 128] f32.

Strategy
--------
Keys at positions >= valid_lens[b] carry zero softmax weight, so only
ceil(vl[b]/128) key-chunks per batch matter.  Scores are O(6), so softmax
needs no max-subtraction and partial (numerator, denominator) sums over
disjoint key ranges are additive -- work splits across cores and is
recombined on the host.

The device program is a flat skew-2 software pipeline over "half-chunks"
(128 keys x 512 queries).  Half-chunks are grouped into slots of 512
queries; slots come in PAIRS that cover the two query-halves of one
(batch, key-range) segment, sharing a single K^T/V input block (halves
the input DMA traffic, which otherwise gates the pipeline start).
Per half-chunk:
  S^T  = K_chunk @ Q^T                 1 matmul -> PSUM [128k x 512q]
  P^T  = exp(SCALE*S^T + mask)         split: ScalarE native exp on 352
         cols, VectorE on 160 cols via a Schraudolph fast-exp (one
         scalar_tensor_tensor producing bf16 *bit patterns* in int16;
         the per-partition multiplier doubles as the key-validity mask)
  PV  += P^T_j^T @ [V_chunk | 1]       4 matmuls (129 cols) accumulating
         in PSUM; ones-column = softmax denominator
Each PV accumulator owns a full PSUM bank (matmul start=True resets the
whole bank, so groups cannot share one): 4 st banks + 4 pv banks.  At
slot end the 4 pv banks are copied (Vector/Scalar alternating) to a bf16
stage tile and DMA'd partition-major (contiguous 1032B rows) so the
output drain overlaps the next slot.

The host schedules (batch, query-half) chunk segments into the pair
grid to balance half-chunks per core (zero-slack when possible), builds
per-core inputs, and sums/normalizes the partial outputs in fp64.
"""

import math
from collections import deque
from contextlib import ExitStack

import ml_dtypes
import numpy as np

import concourse.bacc as bacc
import concourse.mybir as mybir
import concourse.tile as tile
from concourse.bass import AP
from concourse.bass_utils import run_bass_kernel_spmd

N_CORES = 8
B, L, D = 8, 2048, 128
CH = 128          # keys per chunk
WQ = 512          # queries per slot
QT_N = WQ // 128  # PV matmul subtiles per slot (4)
WH = 1024         # queries per pair (two slots)
DV = D + 1        # V columns + ones column
OW = QT_N * DV    # output columns per slot (516)
SCALE = 1.0 / math.sqrt(D)
MASK_BIAS = -100.0

# Schraudolph fast-exp: bf16 bits of exp(z) ~ int16((st + DADD)*CMUL)
# where z = SCALE*st.  CMUL = SCALE * 128/ln2; DADD = (127*128 + ADJ)/CMUL.
# ADJ centers the 2^frac chord (its max overshoot is ~+6%).
A16 = 128.0 / math.log(2.0)
CMUL = A16 * SCALE
ADJ = -5.9
DADD = (16256.0 + ADJ) / CMUL
NS = 352          # ScalarE exp columns per half-chunk
NV = WQ - NS      # VectorE fast-exp columns

BF16 = ml_dtypes.bfloat16


# ---------------------------------------------------------------- scheduling

def _try_pack(groups, structure, order, n_cores):
    """Cut groups (id, nchunks) into segments placed into bins of the given
    structure (one bin per (core, pair)).  Returns {(core, pair): (gid,
    chunk_start, nchunks)} or None if the groups don't fit."""
    bins = []  # [capacity, core, pair]
    for s, c in enumerate(structure):
        for core in range(n_cores):
            bins.append([c, core, s])
    placement = {}
    for gid, total in order:
        done = 0
        while done < total:
            rem = total - done
            if not bins:
                return None
            bins.sort(key=lambda b: b[0])
            if rem >= bins[-1][0]:
                cap, core, s = bins.pop()
            else:
                i = next(i for i, b in enumerate(bins) if b[0] >= rem)
                cap, core, s = bins.pop(i)
            take = min(cap, rem)
            placement[(core, s)] = (gid, done, take)
            done += take
    return placement


def _schedule(valid_lens):
    """Choose a pair structure [C_1..C_P] (identical on every core) and an
    assignment of (batch, query-half) chunk segments to (core, pair).
    Pair p expands to slots 2p (queries qh*1024..+512) and 2p+1 (+512..1024)
    sharing one K/V block."""
    import random

    nk = [max(1, -(-int(v) // CH)) for v in valid_lens]
    groups = []  # gid -> (b, qh, nchunks)
    for b in range(B):
        for qh in range(L // WH):
            groups.append((b, qh, nk[b]))
    sizes = [(gid, g[2]) for gid, g in enumerate(groups)]
    t_all = sum(s for _, s in sizes)
    tpc0 = max(1, -(-t_all // N_CORES))
    rng = random.Random(0)

    def partitions(n, max_parts):
        def rec(n, maxval, parts):
            if n == 0:
                yield list(parts)
                return
            if len(parts) == max_parts:
                return
            for v in range(min(n, maxval), 0, -1):
                parts.append(v)
                yield from rec(n - v, v, parts)
                parts.pop()

        yield from rec(n, n, [])

    best = None  # (tpc, n_pairs, structure, placement)
    for tpc in range(tpc0, tpc0 + 2 * max(nk) + 2):
        for structure in partitions(tpc, 6):
            orders = [sorted(sizes, key=lambda x: -x[1])]
            for _ in range(300):
                o = sizes[:]
                rng.shuffle(o)
                orders.append(o)
            for order in orders:
                placement = _try_pack(groups, structure, order, N_CORES)
                if placement is not None:
                    cand = (tpc, len(structure), structure, placement)
                    if best is None or cand[:2] < best[:2]:
                        best = cand
                    break  # this structure is feasible; try next structure
        if best is not None and best[0] == tpc:
            break  # nothing with fewer chunks/core exists at this point
    assert best is not None
    _, _, structure, placement = best
    passign = [[None] * len(structure) for _ in range(N_CORES)]
    for (core, s), (gid, start, n) in placement.items():
        b, qh, _ = groups[gid]
        passign[core][s] = (b, qh, start, n)
    # ascending pair size: small pairs first (their input lands first, the
    # pipeline starts early) and big pairs last (their bigger inputs have
    # time to arrive while earlier slots compute).
    order = sorted(range(len(structure)), key=lambda s: structure[s])
    structure = [structure[s] for s in order]
    passign = [[row[s] for s in order] for row in passign]
    return structure, passign


# ------------------------------------------------------------- device program

def _pair_layout(structure):
    """Per-pair combined input layout: [qtE | qtO | kt | vx] in one bf16
    buffer.  Returns (offsets, total_width): offsets[p] = (qt_base, kt_off,
    vx_off)."""
    offsets = []
    base = 0
    for C in structure:
        offsets.append((base, base + WH, base + WH + C * CH))
        base += WH + C * (CH + DV)
    return offsets, base


def _build_program(structure):
    P = len(structure)           # pairs
    S = 2 * P                    # slots
    TP = sum(structure)          # chunks per pair column (bias table width)
    T = 2 * TP                   # total half-chunks
    offsets, totw = _pair_layout(structure)
    pair_g0 = np.cumsum([0] + structure[:-1]).tolist()
    slot_g0 = []                 # pt base index per slot
    acc = 0
    for C in structure:
        slot_g0.extend([acc, acc + C])
        acc += 2 * C
    nc = bacc.Bacc("TRN2", target_bir_lowering=False, debug=False)
    data_d = nc.dram_tensor("data", [128, totw], mybir.dt.bfloat16,
                            kind="ExternalInput").ap()
    # tables: [biasS (TP) | dveC (TP) | dveD (1)] fp32, shared by both slots
    # of a pair
    tbl_d = nc.dram_tensor("tbl", [128, 2 * TP + 1], mybir.dt.float32,
                           kind="ExternalInput").ap()
    out_d = nc.dram_tensor("out", [S * 128, OW], mybir.dt.bfloat16,
                           kind="ExternalOutput").ap()

    with tile.TileContext(nc) as tc, ExitStack() as ctx:
        sb_pool = ctx.enter_context(tc.tile_pool(name="sb", bufs=1))
        st_pool = ctx.enter_context(tc.tile_pool(name="st", bufs=4,
                                                 space="PSUM"))
        pv_pool = ctx.enter_context(tc.tile_pool(name="pv", bufs=4,
                                                 space="PSUM"))
        stage_pool = ctx.enter_context(tc.tile_pool(name="stage", bufs=2))

        tbl_sb = sb_pool.tile([128, 2 * TP + 1], mybir.dt.float32)
        nc.scalar.dma_start(tbl_sb[:], tbl_d[:])
        biasS = tbl_sb[:, 0:TP]
        dveC = tbl_sb[:, TP:2 * TP]
        dveD = tbl_sb[:, 2 * TP:2 * TP + 1]

        data_sb = sb_pool.tile([128, totw], mybir.dt.bfloat16)
        pt_sb = sb_pool.tile([128, T * WQ], mybir.dt.bfloat16)

        # warmup memset goes first on the gpsimd queue -- anything queued
        # before it would delay the PE warmup chain.
        warm_sb = sb_pool.tile([128, 512], mybir.dt.bfloat16)
        nc.gpsimd.memset(warm_sb[:], 0.0)

        # input DMAs, all issued upfront in pair order, spread over the three
        # DMA-capable queues so descriptor issue (~0.7us each) parallelizes
        # and transfers land roughly in the order compute consumes them.
        qrr = [nc.sync, nc.gpsimd, nc.scalar]
        qi = 0

        def piece(a, b):
            nonlocal qi
            qrr[qi % 3].dma_start(data_sb[:, a:b], data_d[:, a:b])
            qi += 1

        for p, C in enumerate(structure):
            base, kt0, vx0 = offsets[p]
            end = vx0 + C * DV
            if C <= 2:
                piece(base, end)
            else:
                piece(base, vx0)   # qtE|qtO|kt
                piece(vx0, end)    # vx
        # PE warmup: dummy matmuls during the initial DMA wait ramp the PE
        # clock toward 2.4 GHz before real work.
        warm_ps = st_pool.tile([128, WQ], mybir.dt.float32, tag="st")
        for _ in range(5):
            nc.tensor.matmul(warm_ps[:], warm_sb[:, 0:128], warm_sb[:])

        # flat chunk list for the skew-2 pipeline: (slot, c, first, last)
        chunks = []
        for s in range(S):
            C = structure[s // 2]
            for c in range(C):
                chunks.append((s, c, c == 0, c == C - 1))

        slot_pv = {}
        pending = deque()
        out_q = [nc.sync, nc.gpsimd]

        def emit_front(s, c):
            p = s // 2
            base, kt0, vx0 = offsets[p]
            g = pair_g0[p] + c
            qt = data_sb[:, base + (s % 2) * WQ:base + (s % 2) * WQ + WQ]
            kt = data_sb[:, kt0 + c * CH:kt0 + (c + 1) * CH]
            st = st_pool.tile([128, WQ], mybir.dt.float32, tag="st")
            nc.tensor.matmul(st[:], kt, qt)
            p0 = (slot_g0[s] + c) * WQ
            nc.scalar.activation(pt_sb[:, p0:p0 + NS], st[:, 0:NS],
                                 mybir.ActivationFunctionType.Exp,
                                 bias=biasS[:, g:g + 1], scale=SCALE)
            cm = dveC[:, g:g + 1]
            cbc = AP(cm.tensor, cm.offset, [[cm.ap[0][0], 128], [0, NV]])
            nc.vector.scalar_tensor_tensor(
                pt_sb[:, p0 + NS:p0 + WQ].bitcast(mybir.dt.int16),
                st[:, NS:WQ], dveD, cbc,
                mybir.AluOpType.add, mybir.AluOpType.mult)

        def emit_back(s, c, first, last):
            p = s // 2
            base, kt0, vx0 = offsets[p]
            if s not in slot_pv:
                # one accumulator per PSUM bank: matmul start=True resets the
                # whole bank, so accumulation groups cannot share one
                slot_pv[s] = [pv_pool.tile([128, 512], mybir.dt.float32,
                                           tag="pv", name=f"pv{s}_{j}")
                              for j in range(QT_N)]
            pvs = slot_pv[s]
            p0 = (slot_g0[s] + c) * WQ
            vx = data_sb[:, vx0 + c * DV:vx0 + (c + 1) * DV]
            for j in range(QT_N):
                nc.tensor.matmul(pvs[j][:, 0:DV],
                                 pt_sb[:, p0 + j * 128:p0 + (j + 1) * 128],
                                 vx, start=first, stop=last)
            if last:
                stage = stage_pool.tile([128, OW], mybir.dt.bfloat16)
                for j in range(QT_N):
                    if j % 2 == 0:
                        nc.vector.tensor_copy(stage[:, j * DV:(j + 1) * DV],
                                              pvs[j][:, 0:DV])
                    else:
                        nc.scalar.copy(stage[:, j * DV:(j + 1) * DV],
                                       pvs[j][:, 0:DV])
                out_q[s % 2].dma_start(out_d[s * 128:(s + 1) * 128, :],
                                       stage[:])

        for ch in chunks:
            emit_front(ch[0], ch[1])
            pending.append(ch)
            if len(pending) > 2:
                emit_back(*pending.popleft())
        while pending:
            emit_back(*pending.popleft())
    nc.compile()
    return nc


# ------------------------------------------------------------------- kernel

def _prep_inputs(queries, keys, values, valid_lens, structure, passign):
    TP = sum(structure)
    offsets, totw = _pair_layout(structure)
    pair_g0 = np.cumsum([0] + structure[:-1]).tolist()
    karange = np.arange(CH)
    in_maps = []
    for core in range(N_CORES):
        data = np.zeros((128, totw), dtype=BF16)
        tbl = np.zeros((128, 2 * TP + 1), dtype=np.float32)
        tbl[:, 0:TP] = MASK_BIAS
        tbl[:, 2 * TP] = DADD
        for p, C in enumerate(structure):
            seg = passign[core][p]
            if seg is None:
                continue
            b, qh, cstart, ncr = seg
            base, kt0, vx0 = offsets[p]
            data[:, base:base + WH] = queries[b, qh * WH:(qh + 1) * WH, :].T
            g = pair_g0[p]
            for ci in range(ncr):
                k0 = (cstart + ci) * CH
                data[:, kt0 + ci * CH:kt0 + (ci + 1) * CH] = \
                    keys[b, k0:k0 + CH, :].T
                data[:, vx0 + ci * DV:vx0 + ci * DV + D] = \
                    values[b, k0:k0 + CH, :]
                valid = (k0 + karange) < int(valid_lens[b])
                data[:, vx0 + ci * DV + D] = valid
                tbl[:, g + ci] = np.where(valid, 0.0, MASK_BIAS)
                tbl[:, TP + g + ci] = np.where(valid, CMUL, 0.0)
        in_maps.append({"data": data, "tbl": tbl})
    return in_maps


def _gather(results, structure, passign):
    S = 2 * len(structure)
    num = np.zeros((B, L, D), dtype=np.float64)
    den = np.zeros((B, L), dtype=np.float64)
    for core in range(N_CORES):
        out = np.asarray(results[core]["out"], dtype=np.float64)
        out = out.reshape(S, 128, OW)
        for p in range(len(structure)):
            seg = passign[core][p]
            if seg is None:
                continue
            b, qh, _, _ = seg
            for half in range(2):
                s = 2 * p + half
                for j in range(QT_N):
                    q0 = qh * WH + half * WQ + j * 128
                    rows = slice(q0, q0 + 128)
                    num[b, rows, :] += out[s, :, j * DV:j * DV + D]
                    den[b, rows] += out[s, :, j * DV + D]
    return (num / den[:, :, None]).astype(np.float32)


def kernel(queries, keys, values, valid_lens):
    queries = np.asarray(queries, dtype=np.float32)
    keys = np.asarray(keys, dtype=np.float32)
    values = np.asarray(values, dtype=np.float32)
    valid_lens = np.asarray(valid_lens, dtype=np.int32)

    structure, passign = _schedule(valid_lens)
    nc = _build_program(structure)
    in_maps = _prep_inputs(queries, keys, values, valid_lens, structure,
                           passign)
    res = run_bass_kernel_spmd(nc, in_maps, core_ids=list(range(N_CORES)))
    return _gather(res.results, structure, passign)
